# revision 17
# baseline (speedup 1.0000x reference)
"""Adversarial-embedding kernel for Trainium2 (8 NeuronCores, SPMD).

Computes (per the nn_Adversarial reference):
    L[t]    = gumbel-argmax over non-pad positions of dpadder[t, :]  (key 42)
    A       = emb, with row (t, L[t]) scaled by (1 + EPS / ||emb[t, L[t]]||)
Returns (A, L).

Sharding: data-parallel over tlen across 8 cores. Each core gets a
(256, 32, 1024) f32 shard flattened to (8192, 1024) plus a (256, 1) int32
row-index tensor. On device: one bulk DRAM->DRAM copy (the memory-bound
part), an indirect-DMA gather of the 256 selected rows, norm+scale on-chip,
and an indirect-DMA scatter of the scaled rows into the output.
"""

import functools
import os

import numpy as np

TLEN, BZ, EMB = 2048, 32, 1024
NCORES = 8
SHARD_T = TLEN // NCORES  # 256
ROWS = SHARD_T * BZ  # 8192
EPSILON = 0.05


def _compute_L(dpadder: np.ndarray) -> np.ndarray:
    """Mirror of reference._sample_valid_indices, on host CPU."""
    import jax
    import jax.numpy as jnp

    cpu = jax.devices("cpu")[0]
    with jax.default_device(cpu):
        g = jax.random.gumbel(jax.random.key(42), (TLEN, BZ), dtype=jnp.float32)
        dp = jnp.asarray(dpadder)
        scores = jnp.where(dp != 1, g, -jnp.inf)
        L = jnp.argmax(scores, axis=1)
    return np.asarray(L)


@functools.lru_cache(maxsize=1)
def _build_program(n_copy_chunks: int = 8):
    from concourse import bacc, bass, mybir
    from concourse.tile import TileContext
    from concourse.tile_rust import add_dep_helper

    f32 = mybir.dt.float32
    i32 = mybir.dt.int32

    # Bacc (not plain Bass): its finalize() runs generate_event_semaphores,
    # which legalizes instructions with >1 sync wait (TRN2 allows only one
    # wait per instruction) by splitting through event semaphores.
    nc = bacc.Bacc()
    emb = nc.declare_dram_parameter("emb", [ROWS, EMB], f32, isOutput=False)
    idx = nc.declare_dram_parameter("idx", [SHARD_T, 1], i32, isOutput=False)
    out = nc.declare_dram_parameter("out", [ROWS, EMB], f32, isOutput=True)

    def raw(inst):
        return getattr(inst, "ins", inst)

    with TileContext(nc) as tc:
        with tc.tile_pool(name="sbuf", bufs=1) as pool:
            # Selected-row indices: partition p of column h holds idx[h*128 + p].
            idx_t = pool.tile([128, 2], i32)
            nc.sync.dma_start(out=idx_t[:, 0:1], in_=idx[0:128, :])
            nc.sync.dma_start(out=idx_t[:, 1:2], in_=idx[128:256, :])

            # Bulk copy: out[:] = emb[:], straight DRAM->DRAM. Split into
            # chunks (finer descriptors -> all 16 SDMA engines saturate
            # early) and alternate the two HWDGE rings (SP + ACT issue).
            copy_insts = []
            rows_per = ROWS // n_copy_chunks
            for c in range(n_copy_chunks):
                sl = slice(c * rows_per, (c + 1) * rows_per)
                eng = nc.sync if c % 2 == 0 else nc.scalar
                ci = eng.dma_start(out=out[sl, :], in_=emb[sl, :])
                copy_insts.append(ci)

            # Gather + scale + scatter the 2x128 selected rows.
            scaled = []
            for half in range(2):
                hidx = idx_t[:, half : half + 1]
                rows = pool.tile([128, EMB], f32, tag=f"rows{half}")
                nc.gpsimd.indirect_dma_start(
                    out=rows[:],
                    out_offset=None,
                    in_=emb[:, :],
                    in_offset=bass.IndirectOffsetOnAxis(ap=hidx, axis=0),
                )
                # Engine split keeps the final tensor_scalar's cross-engine
                # waits to one producer (walrus rejects TensorScalarPtr with
                # multiple sync waits): square+reduce and reciprocal on DVE,
                # sqrt and the affine scale on ACT, multiply back on DVE.
                sq = pool.tile([128, EMB], f32, tag=f"sq{half}")
                ss = pool.tile([128, 1], f32, tag=f"ss{half}")
                nc.vector.tensor_tensor(
                    out=sq[:], in0=rows[:], in1=rows[:], op=mybir.AluOpType.mult
                )
                nc.vector.reduce_sum(
                    out=ss[:], in_=sq[:], axis=mybir.AxisListType.X
                )
                rss = pool.tile([128, 1], f32, tag=f"rss{half}")
                nc.vector.reciprocal(out=rss[:], in_=ss[:])  # 1 / sumsq
                rsq = pool.tile([128, 1], f32, tag=f"rsq{half}")
                nc.scalar.sqrt(out=rsq[:], in_=rss[:])  # 1 / norm
                # scale = 1 + EPSILON / norm
                scl = pool.tile([128, 1], f32, tag=f"scl{half}")
                nc.scalar.activation(
                    out=scl[:],
                    in_=rsq[:],
                    func=mybir.ActivationFunctionType.Copy,
                    bias=1.0,
                    scale=EPSILON,
                )
                ts = nc.vector.tensor_scalar_mul(
                    out=rows[:], in0=rows[:], scalar1=scl[:, :1]
                )
                scaled.append((hidx, rows, ts))

            # The scatters must land after the bulk copy of the rows they
            # overwrite (WAW on `out`). Scatter half h covers timesteps
            # [h*128, (h+1)*128) -> emb rows [h*4096, (h+1)*4096), i.e. the
            # h-th half of the chunks — so scatter 0 can fire at copy
            # half-time instead of serializing after the full copy.
            for half, (hidx, rows, _) in enumerate(scaled):
                si = nc.gpsimd.indirect_dma_start(
                    out=out[:, :],
                    out_offset=bass.IndirectOffsetOnAxis(ap=hidx, axis=0),
                    in_=rows[:],
                    in_offset=None,
                )
                lo = half * (ROWS // 2)
                hi = lo + ROWS // 2
                for c, ci in enumerate(copy_insts):
                    if c * rows_per < hi and (c + 1) * rows_per > lo:
                        add_dep_helper(
                            raw(si), raw(ci), reason="scatter after bulk copy"
                        )

    # run_bass_via_pjrt binds the exec primitive without finalizing; Bacc
    # needs finalize() to run its compile() pipeline (register allocation,
    # event-semaphore legalization of multi-wait instructions, ...).
    nc.finalize()
    return nc


def _install_ntff_hook_shim():
    """This image lacks antenv.axon_hooks (which bass_utils imports for
    trace=True under axon); synthesize it from trn_boot's ctypes NTFF
    profiler. Profiling-only — never needed for plain execution."""
    import sys
    import types

    try:
        import antenv.axon_hooks  # noqa: F401

        return
    except ImportError:
        pass
    try:
        import antenv
        from trn_agent_boot.trn_boot import _ntff_profile_via_ctypes
    except ImportError:
        return
    hook = _ntff_profile_via_ctypes("/opt/axon/libaxon_pjrt.so")
    mod = types.ModuleType("antenv.axon_hooks")
    mod.get_axon_ntff_profile_hook = lambda: hook
    mod.set_axon_ntff_profile_hook = lambda h: None
    sys.modules["antenv.axon_hooks"] = mod
    antenv.axon_hooks = mod


def kernel(emb, data, dpadder):
    emb = np.ascontiguousarray(np.asarray(emb), dtype=np.float32)
    dpad = np.asarray(dpadder)
    L = _compute_L(dpad)

    from concourse.bass_utils import run_bass_kernel_spmd

    nc = _build_program()

    t = np.arange(SHARD_T, dtype=np.int64)
    in_maps = []
    for s in range(NCORES):
        Ls = L[s * SHARD_T : (s + 1) * SHARD_T].astype(np.int64)
        rowidx = (t * BZ + Ls).astype(np.int32).reshape(SHARD_T, 1)
        shard = emb[s * SHARD_T : (s + 1) * SHARD_T].reshape(ROWS, EMB)
        in_maps.append({"emb": shard, "idx": rowidx})

    trace = os.environ.get("KERNEL_TRACE", "0") == "1"
    if trace:
        _install_ntff_hook_shim()
    res = run_bass_kernel_spmd(
        nc, in_maps, list(range(NCORES)), trace=trace
    )
    if trace:
        print(f"HW exec time: {res.exec_time_ns} ns")
        print(f"mean exec time: {res.mean_exec_time_ns} ns")
        if res.instructions_and_trace is not None:
            print(f"trace: {res.instructions_and_trace[1]}")

    A = np.concatenate(
        [r["out"].reshape(SHARD_T, BZ, EMB) for r in res.results], axis=0
    )
    L_dtype = np.int64 if np.asarray(data).dtype == np.int64 else np.int32
    return A, L.astype(L_dtype)


# revision 18
# speedup vs baseline: 1.0050x; 1.0050x over previous
"""Adversarial-embedding kernel for Trainium2 (8 NeuronCores, SPMD).

Computes (per the nn_Adversarial reference):
    L[t]    = gumbel-argmax over non-pad positions of dpadder[t, :]  (key 42)
    A       = emb, with row (t, L[t]) scaled by (1 + EPS / ||emb[t, L[t]]||)
Returns (A, L).

Sharding: data-parallel over tlen across 8 cores. Each core gets a
(256, 32, 1024) f32 shard flattened to (8192, 1024) plus a (256, 1) int32
row-index tensor. On device: one bulk DRAM->DRAM copy (the memory-bound
part), an indirect-DMA gather of the 256 selected rows, norm+scale on-chip,
and an indirect-DMA scatter of the scaled rows into the output.
"""

import functools
import os

import numpy as np

TLEN, BZ, EMB = 2048, 32, 1024
NCORES = 8
SHARD_T = TLEN // NCORES  # 256
ROWS = SHARD_T * BZ  # 8192
EPSILON = 0.05


def _compute_L(dpadder: np.ndarray) -> np.ndarray:
    """Mirror of reference._sample_valid_indices, on host CPU."""
    import jax
    import jax.numpy as jnp

    cpu = jax.devices("cpu")[0]
    with jax.default_device(cpu):
        g = jax.random.gumbel(jax.random.key(42), (TLEN, BZ), dtype=jnp.float32)
        dp = jnp.asarray(dpadder)
        scores = jnp.where(dp != 1, g, -jnp.inf)
        L = jnp.argmax(scores, axis=1)
    return np.asarray(L)


@functools.lru_cache(maxsize=1)
def _build_program(n_copy_chunks: int = 16):
    from concourse import bacc, bass, mybir
    from concourse.tile import TileContext
    from concourse.tile_rust import add_dep_helper

    f32 = mybir.dt.float32
    i32 = mybir.dt.int32

    # Bacc (not plain Bass): its finalize() runs generate_event_semaphores,
    # which legalizes instructions with >1 sync wait (TRN2 allows only one
    # wait per instruction) by splitting through event semaphores.
    nc = bacc.Bacc()
    emb = nc.declare_dram_parameter("emb", [ROWS, EMB], f32, isOutput=False)
    idx = nc.declare_dram_parameter("idx", [SHARD_T, 1], i32, isOutput=False)
    out = nc.declare_dram_parameter("out", [ROWS, EMB], f32, isOutput=True)

    def raw(inst):
        return getattr(inst, "ins", inst)

    with TileContext(nc) as tc:
        with tc.tile_pool(name="sbuf", bufs=1) as pool:
            # Selected-row indices: partition p of column h holds idx[h*128 + p].
            idx_t = pool.tile([128, 2], i32)
            nc.sync.dma_start(out=idx_t[:, 0:1], in_=idx[0:128, :])
            nc.sync.dma_start(out=idx_t[:, 1:2], in_=idx[128:256, :])

            # Bulk copy: out[:] = emb[:], straight DRAM->DRAM. Split into
            # chunks (finer descriptors -> all 16 SDMA engines saturate
            # early) and alternate the two HWDGE rings (SP + ACT issue).
            copy_insts = []
            rows_per = ROWS // n_copy_chunks
            for c in range(n_copy_chunks):
                sl = slice(c * rows_per, (c + 1) * rows_per)
                eng = nc.sync if c % 2 == 0 else nc.scalar
                ci = eng.dma_start(out=out[sl, :], in_=emb[sl, :])
                copy_insts.append(ci)

            # Gather + scale + scatter the 2x128 selected rows.
            scaled = []
            for half in range(2):
                hidx = idx_t[:, half : half + 1]
                rows = pool.tile([128, EMB], f32, tag=f"rows{half}")
                nc.gpsimd.indirect_dma_start(
                    out=rows[:],
                    out_offset=None,
                    in_=emb[:, :],
                    in_offset=bass.IndirectOffsetOnAxis(ap=hidx, axis=0),
                )
                # Engine split keeps the final tensor_scalar's cross-engine
                # waits to one producer (walrus rejects TensorScalarPtr with
                # multiple sync waits): square+reduce and reciprocal on DVE,
                # sqrt and the affine scale on ACT, multiply back on DVE.
                sq = pool.tile([128, EMB], f32, tag=f"sq{half}")
                ss = pool.tile([128, 1], f32, tag=f"ss{half}")
                nc.vector.tensor_tensor(
                    out=sq[:], in0=rows[:], in1=rows[:], op=mybir.AluOpType.mult
                )
                nc.vector.reduce_sum(
                    out=ss[:], in_=sq[:], axis=mybir.AxisListType.X
                )
                rss = pool.tile([128, 1], f32, tag=f"rss{half}")
                nc.vector.reciprocal(out=rss[:], in_=ss[:])  # 1 / sumsq
                rsq = pool.tile([128, 1], f32, tag=f"rsq{half}")
                nc.scalar.sqrt(out=rsq[:], in_=rss[:])  # 1 / norm
                # scale = 1 + EPSILON / norm
                scl = pool.tile([128, 1], f32, tag=f"scl{half}")
                nc.scalar.activation(
                    out=scl[:],
                    in_=rsq[:],
                    func=mybir.ActivationFunctionType.Copy,
                    bias=1.0,
                    scale=EPSILON,
                )
                ts = nc.vector.tensor_scalar_mul(
                    out=rows[:], in0=rows[:], scalar1=scl[:, :1]
                )
                scaled.append((hidx, rows, ts))

            # The scatters must land after the bulk copy of the rows they
            # overwrite (WAW on `out`). Scatter half h covers timesteps
            # [h*128, (h+1)*128) -> emb rows [h*4096, (h+1)*4096), i.e. the
            # h-th half of the chunks — so scatter 0 can fire at copy
            # half-time instead of serializing after the full copy.
            for half, (hidx, rows, _) in enumerate(scaled):
                si = nc.gpsimd.indirect_dma_start(
                    out=out[:, :],
                    out_offset=bass.IndirectOffsetOnAxis(ap=hidx, axis=0),
                    in_=rows[:],
                    in_offset=None,
                )
                lo = half * (ROWS // 2)
                hi = lo + ROWS // 2
                for c, ci in enumerate(copy_insts):
                    if c * rows_per < hi and (c + 1) * rows_per > lo:
                        add_dep_helper(
                            raw(si), raw(ci), reason="scatter after bulk copy"
                        )

    # run_bass_via_pjrt binds the exec primitive without finalizing; Bacc
    # needs finalize() to run its compile() pipeline (register allocation,
    # event-semaphore legalization of multi-wait instructions, ...).
    nc.finalize()
    return nc


def _install_ntff_hook_shim():
    """This image lacks antenv.axon_hooks (which bass_utils imports for
    trace=True under axon); synthesize it from trn_boot's ctypes NTFF
    profiler. Profiling-only — never needed for plain execution."""
    import sys
    import types

    try:
        import antenv.axon_hooks  # noqa: F401

        return
    except ImportError:
        pass
    try:
        import antenv
        from trn_agent_boot.trn_boot import _ntff_profile_via_ctypes
    except ImportError:
        return
    hook = _ntff_profile_via_ctypes("/opt/axon/libaxon_pjrt.so")
    mod = types.ModuleType("antenv.axon_hooks")
    mod.get_axon_ntff_profile_hook = lambda: hook
    mod.set_axon_ntff_profile_hook = lambda h: None
    sys.modules["antenv.axon_hooks"] = mod
    antenv.axon_hooks = mod


def kernel(emb, data, dpadder):
    emb = np.ascontiguousarray(np.asarray(emb), dtype=np.float32)
    dpad = np.asarray(dpadder)
    L = _compute_L(dpad)

    from concourse.bass_utils import run_bass_kernel_spmd

    nc = _build_program()

    t = np.arange(SHARD_T, dtype=np.int64)
    in_maps = []
    for s in range(NCORES):
        Ls = L[s * SHARD_T : (s + 1) * SHARD_T].astype(np.int64)
        rowidx = (t * BZ + Ls).astype(np.int32).reshape(SHARD_T, 1)
        shard = emb[s * SHARD_T : (s + 1) * SHARD_T].reshape(ROWS, EMB)
        in_maps.append({"emb": shard, "idx": rowidx})

    trace = os.environ.get("KERNEL_TRACE", "0") == "1"
    if trace:
        _install_ntff_hook_shim()
    res = run_bass_kernel_spmd(
        nc, in_maps, list(range(NCORES)), trace=trace
    )
    if trace:
        print(f"HW exec time: {res.exec_time_ns} ns")
        print(f"mean exec time: {res.mean_exec_time_ns} ns")
        if res.instructions_and_trace is not None:
            print(f"trace: {res.instructions_and_trace[1]}")

    A = np.concatenate(
        [r["out"].reshape(SHARD_T, BZ, EMB) for r in res.results], axis=0
    )
    L_dtype = np.int64 if np.asarray(data).dtype == np.int64 else np.int32
    return A, L.astype(L_dtype)


# revision 21
# speedup vs baseline: 1.0252x; 1.0201x over previous
"""Adversarial-embedding kernel for Trainium2 (8 NeuronCores, SPMD).

Computes (per the nn_Adversarial reference):
    L[t]    = gumbel-argmax over non-pad positions of dpadder[t, :]  (key 42)
    A       = emb, with row (t, L[t]) scaled by (1 + EPS / ||emb[t, L[t]]||)
Returns (A, L).

Sharding: data-parallel over tlen across 8 cores. Each core gets a
(256, 32, 1024) f32 shard flattened to (8192, 1024) plus a (256, 1) int32
row-index tensor. On device: one bulk DRAM->DRAM copy (the memory-bound
part), an indirect-DMA gather of the 256 selected rows, norm+scale on-chip,
and an indirect-DMA scatter of the scaled rows into the output.
"""

import functools
import os

import numpy as np

TLEN, BZ, EMB = 2048, 32, 1024
NCORES = 8
SHARD_T = TLEN // NCORES  # 256
ROWS = SHARD_T * BZ  # 8192
EPSILON = 0.05


def _compute_L(dpadder: np.ndarray) -> np.ndarray:
    """Mirror of reference._sample_valid_indices, on host CPU."""
    import jax
    import jax.numpy as jnp

    cpu = jax.devices("cpu")[0]
    with jax.default_device(cpu):
        g = jax.random.gumbel(jax.random.key(42), (TLEN, BZ), dtype=jnp.float32)
        dp = jnp.asarray(dpadder)
        scores = jnp.where(dp != 1, g, -jnp.inf)
        L = jnp.argmax(scores, axis=1)
    return np.asarray(L)


@functools.lru_cache(maxsize=1)
def _build_program(n_copy_chunks: int = 12):
    from concourse import bacc, bass, mybir
    from concourse.tile import TileContext
    from concourse.tile_rust import add_dep_helper

    f32 = mybir.dt.float32
    i32 = mybir.dt.int32

    # Bacc (not plain Bass): its finalize() runs generate_event_semaphores,
    # which legalizes instructions with >1 sync wait (TRN2 allows only one
    # wait per instruction) by splitting through event semaphores.
    nc = bacc.Bacc()
    emb = nc.declare_dram_parameter("emb", [ROWS, EMB], f32, isOutput=False)
    idx = nc.declare_dram_parameter("idx", [SHARD_T, 1], i32, isOutput=False)
    out = nc.declare_dram_parameter("out", [ROWS, EMB], f32, isOutput=True)

    def raw(inst):
        return getattr(inst, "ins", inst)

    with TileContext(nc) as tc:
        with tc.tile_pool(name="sbuf", bufs=1) as pool:
            # Selected-row indices: partition p of column h holds idx[h*128 + p].
            idx_t = pool.tile([128, 2], i32)
            nc.sync.dma_start(out=idx_t[:, 0:1], in_=idx[0:128, :])
            nc.sync.dma_start(out=idx_t[:, 1:2], in_=idx[128:256, :])

            # Bulk copy: out[:] = emb[:], straight DRAM->DRAM. Split into
            # chunks (finer descriptors -> all 16 SDMA engines saturate
            # early) and alternate the two HWDGE rings (SP + ACT issue).
            copy_insts = []
            rows_per = ROWS // n_copy_chunks
            for c in range(n_copy_chunks):
                sl = slice(c * rows_per, (c + 1) * rows_per)
                eng = nc.sync if c % 2 == 0 else nc.scalar
                ci = eng.dma_start(out=out[sl, :], in_=emb[sl, :])
                copy_insts.append(ci)

            # Gather + scale + scatter the 2x128 selected rows.
            scaled = []
            for half in range(2):
                hidx = idx_t[:, half : half + 1]
                rows = pool.tile([128, EMB], f32, tag=f"rows{half}")
                nc.gpsimd.indirect_dma_start(
                    out=rows[:],
                    out_offset=None,
                    in_=emb[:, :],
                    in_offset=bass.IndirectOffsetOnAxis(ap=hidx, axis=0),
                )
                # Engine split keeps the final tensor_scalar's cross-engine
                # waits to one producer (walrus rejects TensorScalarPtr with
                # multiple sync waits): square+reduce and reciprocal on DVE,
                # sqrt and the affine scale on ACT, multiply back on DVE.
                sq = pool.tile([128, EMB], f32, tag=f"sq{half}")
                ss = pool.tile([128, 1], f32, tag=f"ss{half}")
                nc.vector.tensor_tensor(
                    out=sq[:], in0=rows[:], in1=rows[:], op=mybir.AluOpType.mult
                )
                nc.vector.reduce_sum(
                    out=ss[:], in_=sq[:], axis=mybir.AxisListType.X
                )
                rss = pool.tile([128, 1], f32, tag=f"rss{half}")
                nc.vector.reciprocal(out=rss[:], in_=ss[:])  # 1 / sumsq
                rsq = pool.tile([128, 1], f32, tag=f"rsq{half}")
                nc.scalar.sqrt(out=rsq[:], in_=rss[:])  # 1 / norm
                # scale = 1 + EPSILON / norm
                scl = pool.tile([128, 1], f32, tag=f"scl{half}")
                nc.scalar.activation(
                    out=scl[:],
                    in_=rsq[:],
                    func=mybir.ActivationFunctionType.Copy,
                    bias=1.0,
                    scale=EPSILON,
                )
                ts = nc.vector.tensor_scalar_mul(
                    out=rows[:], in0=rows[:], scalar1=scl[:, :1]
                )
                scaled.append((hidx, rows, ts))

            # The scatters must land after the bulk copy of the rows they
            # overwrite (WAW on `out`). Scatter half h covers timesteps
            # [h*128, (h+1)*128) -> emb rows [h*4096, (h+1)*4096), i.e. the
            # h-th half of the chunks — so scatter 0 can fire at copy
            # half-time instead of serializing after the full copy.
            for half, (hidx, rows, _) in enumerate(scaled):
                si = nc.gpsimd.indirect_dma_start(
                    out=out[:, :],
                    out_offset=bass.IndirectOffsetOnAxis(ap=hidx, axis=0),
                    in_=rows[:],
                    in_offset=None,
                )
                lo = half * (ROWS // 2)
                hi = lo + ROWS // 2
                for c, ci in enumerate(copy_insts):
                    if c * rows_per < hi and (c + 1) * rows_per > lo:
                        add_dep_helper(
                            raw(si), raw(ci), reason="scatter after bulk copy"
                        )

    # run_bass_via_pjrt binds the exec primitive without finalizing; Bacc
    # needs finalize() to run its compile() pipeline (register allocation,
    # event-semaphore legalization of multi-wait instructions, ...).
    nc.finalize()
    return nc


def _install_ntff_hook_shim():
    """This image lacks antenv.axon_hooks (which bass_utils imports for
    trace=True under axon); synthesize it from trn_boot's ctypes NTFF
    profiler. Profiling-only — never needed for plain execution."""
    import sys
    import types

    try:
        import antenv.axon_hooks  # noqa: F401

        return
    except ImportError:
        pass
    try:
        import antenv
        from trn_agent_boot.trn_boot import _ntff_profile_via_ctypes
    except ImportError:
        return
    hook = _ntff_profile_via_ctypes("/opt/axon/libaxon_pjrt.so")
    mod = types.ModuleType("antenv.axon_hooks")
    mod.get_axon_ntff_profile_hook = lambda: hook
    mod.set_axon_ntff_profile_hook = lambda h: None
    sys.modules["antenv.axon_hooks"] = mod
    antenv.axon_hooks = mod


LAST_EXEC_NS = None


def kernel(emb, data, dpadder):
    global LAST_EXEC_NS
    emb = np.ascontiguousarray(np.asarray(emb), dtype=np.float32)
    dpad = np.asarray(dpadder)
    L = _compute_L(dpad)

    from concourse.bass_utils import run_bass_kernel_spmd

    nc = _build_program()

    t = np.arange(SHARD_T, dtype=np.int64)
    in_maps = []
    for s in range(NCORES):
        Ls = L[s * SHARD_T : (s + 1) * SHARD_T].astype(np.int64)
        rowidx = (t * BZ + Ls).astype(np.int32).reshape(SHARD_T, 1)
        shard = emb[s * SHARD_T : (s + 1) * SHARD_T].reshape(ROWS, EMB)
        in_maps.append({"emb": shard, "idx": rowidx})

    trace = os.environ.get("KERNEL_TRACE", "0") == "1"
    if trace:
        _install_ntff_hook_shim()
    res = run_bass_kernel_spmd(
        nc, in_maps, list(range(NCORES)), trace=trace
    )
    if trace:
        LAST_EXEC_NS = res.exec_time_ns
        print(f"HW exec time: {res.exec_time_ns} ns")
        print(f"mean exec time: {res.mean_exec_time_ns} ns")
        if res.instructions_and_trace is not None:
            print(f"trace: {res.instructions_and_trace[1]}")

    A = np.concatenate(
        [r["out"].reshape(SHARD_T, BZ, EMB) for r in res.results], axis=0
    )
    L_dtype = np.int64 if np.asarray(data).dtype == np.int64 else np.int32
    return A, L.astype(L_dtype)


# revision 24
# speedup vs baseline: 1.1820x; 1.1530x over previous
"""Adversarial-embedding kernel for Trainium2 (8 NeuronCores, SPMD).

Computes (per the nn_Adversarial reference):
    L[t]    = gumbel-argmax over non-pad positions of dpadder[t, :]  (key 42)
    A       = emb, with row (t, L[t]) scaled by (1 + EPS / ||emb[t, L[t]]||)
Returns (A, L).

Sharding: data-parallel over tlen across 8 cores. Each core gets a
(256, 32, 1024) f32 shard flattened to (8192, 1024) plus a (256, 1) int32
row-index tensor. On device: one bulk DRAM->DRAM copy (the memory-bound
part), an indirect-DMA gather of the 256 selected rows, norm+scale on-chip,
and an indirect-DMA scatter of the scaled rows into the output.
"""

import functools
import os

import numpy as np

TLEN, BZ, EMB = 2048, 32, 1024
NCORES = 8
SHARD_T = TLEN // NCORES  # 256
ROWS = SHARD_T * BZ  # 8192
EPSILON = 0.05


def _compute_L(dpadder: np.ndarray) -> np.ndarray:
    """Mirror of reference._sample_valid_indices, on host CPU."""
    import jax
    import jax.numpy as jnp

    cpu = jax.devices("cpu")[0]
    with jax.default_device(cpu):
        g = jax.random.gumbel(jax.random.key(42), (TLEN, BZ), dtype=jnp.float32)
        dp = jnp.asarray(dpadder)
        scores = jnp.where(dp != 1, g, -jnp.inf)
        L = jnp.argmax(scores, axis=1)
    return np.asarray(L)


@functools.lru_cache(maxsize=1)
def _build_program(n_copy_chunks: int = 16):
    from concourse import bacc, bass, mybir
    from concourse.tile import TileContext
    from concourse.tile_rust import add_dep_helper

    f32 = mybir.dt.float32
    i32 = mybir.dt.int32

    # Bacc (not plain Bass): its finalize() runs generate_event_semaphores,
    # which legalizes instructions with >1 sync wait (TRN2 allows only one
    # wait per instruction) by splitting through event semaphores.
    nc = bacc.Bacc()
    emb = nc.declare_dram_parameter("emb", [ROWS, EMB], f32, isOutput=False)
    idx = nc.declare_dram_parameter("idx", [SHARD_T, 1], i32, isOutput=False)
    out = nc.declare_dram_parameter("out", [ROWS, EMB], f32, isOutput=True)

    def raw(inst):
        return getattr(inst, "ins", inst)

    with TileContext(nc) as tc:
        with tc.tile_pool(name="sbuf", bufs=1) as pool:
            # Selected-row indices: partition p of column h holds idx[h*128 + p].
            idx_t = pool.tile([128, 2], i32)
            nc.sync.dma_start(out=idx_t[:, 0:1], in_=idx[0:128, :])
            nc.sync.dma_start(out=idx_t[:, 1:2], in_=idx[128:256, :])

            # Bulk copy: out[:] = emb[:], straight DRAM->DRAM. Split into
            # chunks (finer descriptors -> all 16 SDMA engines saturate
            # early) and alternate the two HWDGE rings (SP + ACT issue).
            copy_insts = []
            rows_per = ROWS // n_copy_chunks
            for c in range(n_copy_chunks):
                sl = slice(c * rows_per, (c + 1) * rows_per)
                eng = nc.sync if c % 2 == 0 else nc.scalar
                ci = eng.dma_start(out=out[sl, :], in_=emb[sl, :])
                copy_insts.append(ci)

            # Gather + scale + scatter the 2x128 selected rows.
            scaled = []
            for half in range(2):
                hidx = idx_t[:, half : half + 1]
                rows = pool.tile([128, EMB], f32, tag=f"rows{half}")
                nc.gpsimd.indirect_dma_start(
                    out=rows[:],
                    out_offset=None,
                    in_=emb[:, :],
                    in_offset=bass.IndirectOffsetOnAxis(ap=hidx, axis=0),
                )
                # All compute on DVE (using ACT would pull in LoadActFuncSet
                # table loads that gate the ACT-ring copy chunks, and keeps
                # every op's cross-engine waits trivial): square + reduce for
                # sumsq, then rsqrt via the int32 bit-hack seed + 3 Newton
                # iterations (fp32-accurate), then scale = 1 + eps*rsqrt.
                sq = pool.tile([128, EMB], f32, tag=f"sq{half}")
                ss = pool.tile([128, 1], f32, tag=f"ss{half}")
                nc.vector.tensor_tensor(
                    out=sq[:], in0=rows[:], in1=rows[:], op=mybir.AluOpType.mult
                )
                nc.vector.reduce_sum(
                    out=ss[:], in_=sq[:], axis=mybir.AxisListType.X
                )
                # z0 = bitcast_f32(0x5f3759df - (bitcast_i32(ss) >> 1))
                zi = pool.tile([128, 1], i32, tag=f"zi{half}")
                nc.vector.tensor_scalar(
                    out=zi[:],
                    in0=ss[:].bitcast(i32),
                    scalar1=1,
                    scalar2=None,
                    op0=mybir.AluOpType.arith_shift_right,
                )
                # zi = ~(zi) + (0x5f3759df + 1)  ==  0x5f3759df - zi
                # (bitwise and arith ops can't share one TensorScalar)
                nc.vector.tensor_scalar(
                    out=zi[:],
                    in0=zi[:],
                    scalar1=-1,
                    scalar2=None,
                    op0=mybir.AluOpType.bitwise_xor,
                )
                nc.vector.tensor_scalar(
                    out=zi[:],
                    in0=zi[:],
                    scalar1=0x5F3759DF + 1,
                    scalar2=None,
                    op0=mybir.AluOpType.add,
                )
                z = zi[:].bitcast(f32)
                t0 = pool.tile([128, 1], f32, tag=f"t0{half}")
                for _ in range(3):
                    # z <- z * (1.5 - 0.5 * ss * z * z)
                    nc.vector.tensor_tensor(
                        out=t0[:], in0=z, in1=z, op=mybir.AluOpType.mult
                    )
                    nc.vector.tensor_tensor(
                        out=t0[:], in0=t0[:], in1=ss[:], op=mybir.AluOpType.mult
                    )
                    nc.vector.tensor_scalar(
                        out=t0[:],
                        in0=t0[:],
                        scalar1=-0.5,
                        scalar2=1.5,
                        op0=mybir.AluOpType.mult,
                        op1=mybir.AluOpType.add,
                    )
                    nc.vector.tensor_tensor(
                        out=zi[:].bitcast(f32), in0=z, in1=t0[:],
                        op=mybir.AluOpType.mult,
                    )
                # scale = 1 + EPSILON * rsqrt(sumsq)
                scl = pool.tile([128, 1], f32, tag=f"scl{half}")
                nc.vector.tensor_scalar(
                    out=scl[:],
                    in0=z,
                    scalar1=EPSILON,
                    scalar2=1.0,
                    op0=mybir.AluOpType.mult,
                    op1=mybir.AluOpType.add,
                )
                ts = nc.vector.tensor_scalar_mul(
                    out=rows[:], in0=rows[:], scalar1=scl[:, :1]
                )
                scaled.append((hidx, rows, ts))

            # The scatters must land after the bulk copy of the rows they
            # overwrite (WAW on `out`). Scatter half h covers timesteps
            # [h*128, (h+1)*128) -> emb rows [h*4096, (h+1)*4096), i.e. the
            # h-th half of the chunks — so scatter 0 can fire at copy
            # half-time instead of serializing after the full copy.
            for half, (hidx, rows, _) in enumerate(scaled):
                si = nc.gpsimd.indirect_dma_start(
                    out=out[:, :],
                    out_offset=bass.IndirectOffsetOnAxis(ap=hidx, axis=0),
                    in_=rows[:],
                    in_offset=None,
                )
                lo = half * (ROWS // 2)
                hi = lo + ROWS // 2
                for c, ci in enumerate(copy_insts):
                    if c * rows_per < hi and (c + 1) * rows_per > lo:
                        add_dep_helper(
                            raw(si), raw(ci), reason="scatter after bulk copy"
                        )

    # run_bass_via_pjrt binds the exec primitive without finalizing; Bacc
    # needs finalize() to run its compile() pipeline (register allocation,
    # event-semaphore legalization of multi-wait instructions, ...).
    nc.finalize()
    return nc


def _install_ntff_hook_shim():
    """This image lacks antenv.axon_hooks (which bass_utils imports for
    trace=True under axon); synthesize it from trn_boot's ctypes NTFF
    profiler. Profiling-only — never needed for plain execution."""
    import sys
    import types

    try:
        import antenv.axon_hooks  # noqa: F401

        return
    except ImportError:
        pass
    try:
        import antenv
        from trn_agent_boot.trn_boot import _ntff_profile_via_ctypes
    except ImportError:
        return
    hook = _ntff_profile_via_ctypes("/opt/axon/libaxon_pjrt.so")
    mod = types.ModuleType("antenv.axon_hooks")
    mod.get_axon_ntff_profile_hook = lambda: hook
    mod.set_axon_ntff_profile_hook = lambda h: None
    sys.modules["antenv.axon_hooks"] = mod
    antenv.axon_hooks = mod


LAST_EXEC_NS = None


def kernel(emb, data, dpadder):
    global LAST_EXEC_NS
    emb = np.ascontiguousarray(np.asarray(emb), dtype=np.float32)
    dpad = np.asarray(dpadder)
    L = _compute_L(dpad)

    from concourse.bass_utils import run_bass_kernel_spmd

    nc = _build_program()

    t = np.arange(SHARD_T, dtype=np.int64)
    in_maps = []
    for s in range(NCORES):
        Ls = L[s * SHARD_T : (s + 1) * SHARD_T].astype(np.int64)
        rowidx = (t * BZ + Ls).astype(np.int32).reshape(SHARD_T, 1)
        shard = emb[s * SHARD_T : (s + 1) * SHARD_T].reshape(ROWS, EMB)
        in_maps.append({"emb": shard, "idx": rowidx})

    trace = os.environ.get("KERNEL_TRACE", "0") == "1"
    if trace:
        _install_ntff_hook_shim()
    res = run_bass_kernel_spmd(
        nc, in_maps, list(range(NCORES)), trace=trace
    )
    if trace:
        LAST_EXEC_NS = res.exec_time_ns
        print(f"HW exec time: {res.exec_time_ns} ns")
        print(f"mean exec time: {res.mean_exec_time_ns} ns")
        if res.instructions_and_trace is not None:
            print(f"trace: {res.instructions_and_trace[1]}")

    A = np.concatenate(
        [r["out"].reshape(SHARD_T, BZ, EMB) for r in res.results], axis=0
    )
    L_dtype = np.int64 if np.asarray(data).dtype == np.int64 else np.int32
    return A, L.astype(L_dtype)
